# revision 1
# baseline (speedup 1.0000x reference)
"""Trainium2 Bass kernel for nn_EnhancedBTIANet (retrieval_knn), 8 NeuronCores.

Sharding: batch-parallel middle pipeline (core c owns rows [128c,128c+128));
N-sharded answer bank + open head (core c owns columns [6250c, 6250c+6250)).
Collectives: AllGather of q_hat^T / hidden^T, AllToAll top-k candidate merge.
Middle activations kept transposed [d, b]; weights host-pre-transposed to
[d_in, d_out]; LN stats free-major via ones-matmuls; biases fused into the
per-partition ACT evacuation of each transposed matmul.
Exact top-10 per row: flat max/max_index/match_replace on each 3125-wide
half-slice (positions are in-slice indices), merged via threshold +
prefix-scan + local_scatter (u16; 50000 < 65536), embeddings gathered with
per-partition indirect DMA from the natural bank.
"""
import sys

for _p in ("/opt/trn_rl_repo", "/opt/trn_rl_repo/concourse"):
    if _p not in sys.path:
        sys.path.insert(0, _p)

import numpy as np

F32 = U16 = U32 = I16 = AF = ALU = None  # populated in _lazy_imports
bass = bacc = mybir = tile = masks = None

NC = 8
B, D, H, KTOP, NANS = 1024, 768, 8, 10, 50000
BL = B // NC
NS = NANS // NC
NQ = 10
QW = NS // NQ  # 625
DK = D // 128
DH = D // H
LN_EPS = 1e-5
NEG = -1e30
NCHUNKS = [512, 113]
NCHOFF = [0, 512]


def _lazy_imports():
    global bass, bacc, mybir, tile, masks, F32, U16, U32, I16, AF, ALU
    import concourse.bass as _bass
    import concourse.bacc as _bacc
    import concourse.mybir as _mybir
    import concourse.tile as _tile
    from concourse import masks as _masks
    bass, bacc, mybir, tile, masks = _bass, _bacc, _mybir, _tile, _masks
    F32 = mybir.dt.float32
    U16 = mybir.dt.uint16
    U32 = mybir.dt.uint32
    I16 = mybir.dt.int16
    AF = mybir.ActivationFunctionType
    ALU = mybir.AluOpType


def build_program(fake_coll=False):
    _lazy_imports()
    nc = bacc.Bacc("TRN2", target_bir_lowering=False, debug=False,
                   num_devices=NC)
    dt = lambda n, s, d=None, k="ExternalInput": nc.dram_tensor(
        n, s, d or F32, kind=k).ap()

    vis_d = dt("vis", [BL, D])
    txt_d = dt("txt", [BL, D])
    ans_d = dt("ans", [NANS, D])
    ansT_d = dt("ansT", [D, NS])
    w2T_d = dt("w2T", [D, NS])
    b2_d = dt("b2", [1, NS])
    noff_d = dt("noff", [128, NQ])
    vqa_wvT_d = dt("vqa_wvT", [D, D]); vqa_outT_d = dt("vqa_outT", [D, D])
    fprojT_d = dt("fprojT", [2 * D, D]); simT_d = dt("simT", [D, D])
    mha_wvT_d = [dt(f"m{i}_wvT", [D, D]) for i in range(5)]
    mha_outT_d = [dt(f"m{i}_outT", [D, D]) for i in range(5)]
    wq4T_d = dt("wq4T", [D, D]); wk4T_d = dt("wk4T", [D, D])
    ffn1T_d = dt("ffn1T", [D, 4 * D]); ffn2T_d = dt("ffn2T", [4 * D, D])
    outpT_d = dt("outpT", [D, D]); open1T_d = dt("open1T", [D, D])
    bias_d = dt("biaspack", [128, 72])
    mbias_d = dt("mbiaspack", [128, 60])
    lng_d = dt("lng", [128, 4 * DK]); lnb_d = dt("lnb", [128, 4 * DK])
    flng_d = dt("flng", [128, DK]); flnb_d = dt("flnb", [128, DK])
    bq4_d = dt("bq4", [1, D]); bk4_d = dt("bk4", [1, D]); bv4_d = dt("bv4", [1, D])
    out_d = dt("out_slice", [B, NS], F32, k="ExternalOutput")

    with tile.TileContext(nc) as tc:
        from contextlib import ExitStack
        es = ExitStack()
        pool = es.enter_context(tc.tile_pool(name="sb", bufs=1))
        mid = es.enter_context(tc.tile_pool(name="mid", bufs=1))
        wpool = es.enter_context(tc.tile_pool(name="wstream", bufs=1))
        wkpool = es.enter_context(tc.tile_pool(name="wkp", bufs=3))
        w2pool = es.enter_context(tc.tile_pool(name="w2p", bufs=1))
        big = es.enter_context(tc.tile_pool(name="big", bufs=1))
        bankp = es.enter_context(tc.tile_pool(name="bankp", bufs=2))
        sc = es.enter_context(tc.tile_pool(name="scores", bufs=2))
        psA = es.enter_context(tc.tile_pool(name="psA", bufs=4, space="PSUM"))
        psB = es.enter_context(tc.tile_pool(name="psB", bufs=3, space="PSUM"))
        dram = es.enter_context(tc.tile_pool(name="dram", bufs=1, space="DRAM"))

        ident = pool.tile([128, 128], F32)
        masks.make_identity(nc, ident[:])
        ones_col = pool.tile([128, 1], F32)
        nc.vector.memset(ones_col[:], 1.0)
        ones_row = pool.tile([1, 128], F32)
        nc.vector.memset(ones_row[:], 1.0)
        biasp = pool.tile([128, 72], F32); nc.sync.dma_start(biasp[:], bias_d)
        mbias = pool.tile([128, 60], F32); nc.sync.dma_start(mbias[:], mbias_d)
        lng = pool.tile([128, 4 * DK], F32); nc.sync.dma_start(lng[:], lng_d)
        lnb = pool.tile([128, 4 * DK], F32); nc.sync.dma_start(lnb[:], lnb_d)
        flng = pool.tile([128, DK], F32); nc.sync.dma_start(flng[:], flng_d)
        flnb = pool.tile([128, DK], F32); nc.sync.dma_start(flnb[:], flnb_d)
        noff = pool.tile([128, NQ], F32); nc.sync.dma_start(noff[:], noff_d)
        eps_t = pool.tile([1, 1], F32)
        nc.vector.memset(eps_t[:], LN_EPS)
        bq4 = pool.tile([1, D], F32); nc.sync.dma_start(bq4[:], bq4_d)
        bk4 = pool.tile([1, D], F32); nc.sync.dma_start(bk4[:], bk4_d)
        bv4 = pool.tile([1, D], F32); nc.sync.dma_start(bv4[:], bv4_d)

        def mmT(outT, wT_dram, xT, nk, ndout, bias_sb=None, func=None):
            """Y^T chunk j = (wT col-chunk j stationary).T-chain over k of xT.
            outT [128, ndout, b]; xT [128, nk, b]; bias per-partition fused."""
            f = func if func is not None else AF.Identity
            b = xT.shape[2]
            for j in range(ndout):
                ps = psA.tile([128, b], F32, tag="mm")
                w_sb = wkpool.tile([128, 6, 128], F32, tag="wj")
                nc.sync.dma_start(
                    w_sb[:, :nk if nk <= 6 else 6, :],
                    wT_dram[0:min(nk, 6) * 128, j * 128:(j + 1) * 128]
                    .rearrange("(c p) n -> p c n", p=128))
                if nk > 6:
                    w_sb2 = w2pool.tile([128, 18, 128], F32, tag="wj2")
                    nc.sync.dma_start(
                        w_sb2[:, :nk - 6, :],
                        wT_dram[6 * 128:nk * 128, j * 128:(j + 1) * 128]
                        .rearrange("(c p) n -> p c n", p=128))
                for k in range(nk):
                    wk_ap = (w_sb[:, k, :] if k < 6 else w_sb2[:, k - 6, :])
                    nc.tensor.matmul(ps[:], wk_ap, xT[:, k, :],
                                     start=(k == 0), stop=(k == nk - 1))
                bias = bias_sb[:, j:j + 1] if bias_sb is not None else 0.0
                nc.scalar.activation(outT[:, j, :], ps[:], f, bias=bias)

        def mmA(out_ap_of, xT, wT_dram, bias_row, dout=D, func=None):
            """orientation A: out[b, j:j+w] = x@W.T + bias; lhsT = xT chunks
            stationary; rhs streamed; bias via K=1 ones-row matmul."""
            f = func if func is not None else AF.Copy
            j = 0
            while j < dout:
                w = min(512, dout - j)
                ps = psA.tile([128, 512], F32, tag="mm")
                for k in range(DK):
                    wsb = wkpool.tile([128, 512], F32, tag="wAk")
                    nc.sync.dma_start(
                        wsb[:, :w], wT_dram[k * 128:(k + 1) * 128, j:j + w])
                    nc.tensor.matmul(ps[:, :w], xT[:, k, :], wsb[:, :w],
                                     start=(k == 0), stop=False)
                nc.tensor.matmul(ps[:, :w], ones_row[0:1, 0:BL],
                                 bias_row[:, j:j + w], start=False, stop=True)
                nc.scalar.activation(out_ap_of(j, w), ps[:, :w], f)
                j += w

        def mmA_multi(pairs, wT_dram, bias_row, dout=D):
            """orientation A for several (xT, out_ap_of) pairs sharing each
            streamed weight chunk — kills redundant weight DMA."""
            j = 0
            while j < dout:
                w = min(512, dout - j)
                pss = []
                for _i in range(len(pairs)):
                    ps_i = psA.tile([128, 512], F32, tag="mm")
                    pss.append(ps_i)
                for k in range(DK):
                    wsb = wkpool.tile([128, 512], F32, tag="wAk")
                    nc.sync.dma_start(
                        wsb[:, :w], wT_dram[k * 128:(k + 1) * 128, j:j + w])
                    for i, (xT, _) in enumerate(pairs):
                        nc.tensor.matmul(pss[i][:, :w], xT[:, k, :],
                                         wsb[:, :w], start=(k == 0),
                                         stop=False)
                for i, (_, out_ap_of) in enumerate(pairs):
                    nc.tensor.matmul(pss[i][:, :w], ones_row[0:1, 0:BL],
                                     bias_row[:, j:j + w], start=False,
                                     stop=True)
                    nc.scalar.activation(out_ap_of(j, w), pss[i][:, :w],
                                         AF.Copy)
                j += w

        def transpose_in(x_dram, name):
            nat = mid.tile([BL, D], F32, tag="g4")
            nc.sync.dma_start(nat[:], x_dram)
            xT = mid.tile([128, DK, BL], F32, tag=f"T_{name}")
            for k in range(DK):
                ps = psB.tile([128, BL], F32, tag="ps1")
                nc.tensor.transpose(ps[:], nat[:, k * 128:(k + 1) * 128],
                                    ident[:])
                nc.scalar.copy(xT[:, k, :], ps[:])
            return xT

        def ln_T(xT, g_ap, b_ap, otag):
            sq = mid.tile([128, DK, BL], F32, tag="ln_sq")
            nc.scalar.activation(sq[:], xT[:], AF.Square)
            s1 = psB.tile([1, BL], F32, tag="ps1")
            for k in range(DK):
                nc.tensor.matmul(s1[:], ones_col[:], xT[:, k, :],
                                 start=(k == 0), stop=(k == DK - 1))
            s2 = psB.tile([1, BL], F32, tag="ps1")
            for k in range(DK):
                nc.tensor.matmul(s2[:], ones_col[:], sq[:, k, :],
                                 start=(k == 0), stop=(k == DK - 1))
            mu = mid.tile([1, BL], F32, tag="ln_mu")
            nc.scalar.activation(mu[:], s1[:], AF.Copy, scale=1.0 / D)
            m2 = mid.tile([1, BL], F32, tag="ln_m2")
            nc.scalar.activation(m2[:], s2[:], AF.Copy, scale=1.0 / D)
            var = mid.tile([1, BL], F32, tag="ln_var")
            nc.vector.tensor_mul(var[:], mu[:], mu[:])
            nc.vector.tensor_sub(var[:], m2[:], var[:])
            std = mid.tile([1, BL], F32, tag="ln_std")
            nc.scalar.activation(std[:], var[:], AF.Sqrt, bias=eps_t[0:1, 0:1])
            rstd = mid.tile([1, BL], F32, tag="ln_rstd")
            nc.vector.reciprocal(rstd[:], std[:])
            bc1 = psB.tile([128, BL], F32, tag="ps1")
            nc.tensor.matmul(bc1[:], ones_row[:], mu[:])
            mub = mid.tile([128, BL], F32, tag="ln_mub")
            nc.scalar.copy(mub[:], bc1[:])
            bc2 = psB.tile([128, BL], F32, tag="ps1")
            nc.tensor.matmul(bc2[:], ones_row[:], rstd[:])
            rstdb = mid.tile([128, BL], F32, tag="ln_rstdb")
            nc.scalar.copy(rstdb[:], bc2[:])
            yT = mid.tile([128, DK, BL], F32, tag=otag)
            nc.vector.tensor_sub(
                yT[:], xT[:],
                mub[:].rearrange("p b -> p () b").broadcast_to([128, DK, BL]))
            nc.vector.tensor_mul(
                yT[:], yT[:],
                rstdb[:].rearrange("p b -> p () b").broadcast_to([128, DK, BL]))
            for k in range(DK):
                nc.scalar.activation(yT[:, k, :], yT[:, k, :], AF.Identity,
                                     bias=b_ap[:, k:k + 1],
                                     scale=g_ap[:, k:k + 1])
            return yT

        # ---------------- middle pipeline up to q_hat ----------------
        visT = transpose_in(vis_d, "vis")
        txtT = transpose_in(txt_d, "txt")

        vqvT = mid.tile([128, DK, BL], F32, tag="g1")
        mmT(vqvT, vqa_wvT_d, visT, DK, DK, bias_sb=biasp[:, 66:72])
        attn_qT = mid.tile([128, DK, BL], F32, tag="g2")
        mmT(attn_qT, vqa_outT_d, vqvT, DK, DK, bias_sb=biasp[:, 0:6])
        catT = mid.tile([128, 2 * DK, BL], F32, tag="bigmid")
        nc.vector.tensor_copy(catT[:, 0:DK, :], visT[:])
        nc.vector.tensor_copy(catT[:, DK:2 * DK, :], attn_qT[:])
        fusedT = mid.tile([128, DK, BL], F32, tag="g1")
        mmT(fusedT, fprojT_d, catT, 2 * DK, DK, bias_sb=biasp[:, 6:12])
        flnT = ln_T(fusedT, flng[:], flnb[:], "g2")
        nc.scalar.activation(flnT[:], flnT[:], AF.Gelu)
        fpT = mid.tile([128, DK, BL], F32, tag="g1")
        mmT(fpT, simT_d, flnT, DK, DK, bias_sb=biasp[:, 12:18])
        fpsq = mid.tile([128, DK, BL], F32, tag="ln_sq")
        nc.scalar.activation(fpsq[:], fpT[:], AF.Square)
        qn = psB.tile([1, BL], F32, tag="ps1")
        for k in range(DK):
            nc.tensor.matmul(qn[:], ones_col[:], fpsq[:, k, :],
                             start=(k == 0), stop=(k == DK - 1))
        qs = mid.tile([1, BL], F32, tag="qs")
        nc.scalar.activation(qs[:], qn[:], AF.Sqrt)
        qr = mid.tile([1, BL], F32, tag="qr")
        nc.vector.reciprocal(qr[:], qs[:])
        qbc = psB.tile([128, BL], F32, tag="ps1")
        nc.tensor.matmul(qbc[:], ones_row[:], qr[:])
        qrb = mid.tile([128, BL], F32, tag="ln_mub")
        nc.scalar.copy(qrb[:], qbc[:])
        qhatT_loc = mid.tile([128, DK, BL], F32, tag="g3")
        nc.vector.tensor_mul(
            qhatT_loc[:], fpT[:],
            qrb[:].rearrange("p b -> p () b").broadcast_to([128, DK, BL]))

        # ---------------- AllGather q_hat^T ----------------
        qag_in = dram.tile([128, DK * BL], F32)
        qag_out = dram.tile([NC, 128, DK * BL], F32)
        nc.gpsimd.dma_start(qag_in[:],
                            qhatT_loc[:].rearrange("p c b -> p (c b)"))
        if fake_coll:
            for r in range(NC):
                nc.gpsimd.dma_start(qag_out[r], qag_in[:])
        else:
            nc.gpsimd.collective_compute(
                "AllGather", ALU.bypass, replica_groups=[list(range(NC))],
                ins=[qag_in.opt()], outs=[qag_out.opt()])
        qhatT = big.tile([128, DK, B], F32, tag="actT_full")
        for k in range(DK):
            nc.sync.dma_start(
                qhatT[:, k, :].rearrange("p (r b) -> p r b", r=NC),
                qag_out[:].rearrange("r p (c b) -> p c r b", c=DK)[:, k])

        # -------- CMF pre-attention (fills DMA gaps of sim phase) --------
        def mha1(i, srcT, otag):
            vT = mid.tile([128, DK, BL], F32, tag="g6")
            mmT(vT, mha_wvT_d[i], srcT, DK, DK,
                bias_sb=mbias[:, i * 12:i * 12 + 6])
            oT = mid.tile([128, DK, BL], F32, tag=otag)
            mmT(oT, mha_outT_d[i], vT, DK, DK,
                bias_sb=mbias[:, i * 12 + 6:i * 12 + 12])
            return oT

        m0T = mha1(0, visT, "mo1")
        r0 = mid.tile([128, DK, BL], F32, tag="g4")
        nc.vector.tensor_add(r0[:], visT[:], m0T[:])
        v1T = ln_T(r0, lng[:, 0:DK], lnb[:, 0:DK], "g7")
        m1T = mha1(1, txtT, "mo1")
        r1 = mid.tile([128, DK, BL], F32, tag="g4")
        nc.vector.tensor_add(r1[:], txtT[:], m1T[:])
        t1T = ln_T(r1, lng[:, DK:2 * DK], lnb[:, DK:2 * DK], "t1T")
        m2T = mha1(2, t1T, "mo1")
        m3T = mha1(3, v1T, "mo2")
        r2 = mid.tile([128, DK, BL], F32, tag="g4")
        nc.vector.tensor_add(r2[:], m2T[:], m3T[:])
        fzT = ln_T(r2, lng[:, 2 * DK:3 * DK], lnb[:, 2 * DK:3 * DK], "fzT")

        # ---------------- sim + top-k candidates ----------------
        cand_v = pool.tile([128, NC, NQ, 8], F32)
        cand_i = pool.tile([128, NC, NQ, 8], F32)

        for h in range(NQ):
            a_sb = bankp.tile([128, DK, QW], F32, tag="bank_q")
            for k in range(DK):
                nc.sync.dma_start(
                    a_sb[:, k, :],
                    ansT_d[k * 128:(k + 1) * 128, h * QW:(h + 1) * QW])
            for j in range(2):
                w = NCHUNKS[j]; o = NCHOFF[j]
                nps = psB.tile([1, 512], F32, tag="ps1")
                for k in range(DK):
                    sq = wpool.tile([128, 512], F32, tag="sqc")
                    nc.scalar.activation(sq[:, :w], a_sb[:, k, o:o + w],
                                         AF.Square)
                    nc.tensor.matmul(nps[:, :w], ones_col[:], sq[:, :w],
                                     start=(k == 0), stop=(k == DK - 1))
                nrm = wpool.tile([1, 512], F32, tag="nrm")
                nc.scalar.activation(nrm[:, :w], nps[:, :w], AF.Sqrt)
                rinv = wpool.tile([1, 512], F32, tag="rinvc")
                nc.vector.reciprocal(rinv[:, :w], nrm[:, :w])
                rbp = psB.tile([128, 512], F32, tag="ps1")
                nc.tensor.matmul(rbp[:, :w], ones_row[:], rinv[:, :w])
                rb = wpool.tile([128, 512], F32, tag="rbsb")
                nc.scalar.copy(rb[:, :w], rbp[:, :w])
                for k in range(DK):
                    nc.vector.tensor_mul(a_sb[:, k, o:o + w],
                                         a_sb[:, k, o:o + w], rb[:, :w])
            for m in range(NC):
                scores = sc.tile([128, QW], F32, tag="scores")
                for j in range(2):
                    w = NCHUNKS[j]; o = NCHOFF[j]
                    sps = psA.tile([128, 512], F32, tag="mm")
                    for k in range(DK):
                        nc.tensor.matmul(
                            sps[:, :w], qhatT[:, k, m * BL:(m + 1) * BL],
                            a_sb[:, k, o:o + w],
                            start=(k == 0), stop=(k == DK - 1))
                    nc.scalar.copy(scores[:, o:o + w], sps[:, :w])
                v8a = pool.tile([128, 8], F32, tag="v8a")
                i8a = pool.tile([128, 8], U16, tag="i8a")
                nc.vector.max(v8a[:], scores[:])
                nc.vector.max_index(i8a[:], v8a[:], scores[:])
                nc.vector.tensor_copy(cand_v[:, m, h, :], v8a[:])
                i8f = pool.tile([128, 8], F32, tag="i8f")
                nc.vector.tensor_copy(i8f[:], i8a[:])
                nc.vector.tensor_scalar(cand_i[:, m, h, :], i8f[:],
                                        noff[:, h:h + 1], scalar2=None,
                                        op0=ALU.add)

        # ---------------- AllToAll candidate merge ----------------
        a2a_vi = dram.tile([NC, 128, 80], F32)
        a2a_vo = dram.tile([NC, 128, 80], F32)
        a2a_ii = dram.tile([NC, 128, 80], F32)
        a2a_io = dram.tile([NC, 128, 80], F32)
        nc.gpsimd.dma_start(a2a_vi[:].rearrange("m p k -> p m k"),
                            cand_v[:].rearrange("p m h k -> p m (h k)"))
        nc.gpsimd.dma_start(a2a_ii[:].rearrange("m p k -> p m k"),
                            cand_i[:].rearrange("p m h k -> p m (h k)"))
        if fake_coll:
            nc.gpsimd.dma_start(a2a_vo[:], a2a_vi[:])
            nc.gpsimd.dma_start(a2a_io[:], a2a_ii[:])
        else:
            nc.gpsimd.collective_compute(
                "AllToAll", ALU.bypass, replica_groups=[list(range(NC))],
                ins=[a2a_vi.opt()], outs=[a2a_vo.opt()])
            nc.gpsimd.collective_compute(
                "AllToAll", ALU.bypass, replica_groups=[list(range(NC))],
                ins=[a2a_ii.opt()], outs=[a2a_io.opt()])
        mg_v = pool.tile([128, 640], F32)
        mg_i = pool.tile([128, 640], F32)
        nc.sync.dma_start(mg_v[:].rearrange("p (r k) -> p r k", r=NC),
                          a2a_vo[:].rearrange("r p k -> p r k"))
        nc.sync.dma_start(mg_i[:].rearrange("p (r k) -> p r k", r=NC),
                          a2a_io[:].rearrange("r p k -> p r k"))

        mv8a = pool.tile([128, 8], F32)
        mrep = pool.tile([128, 640], F32)
        mv8b = pool.tile([128, 8], F32)
        nc.vector.max(mv8a[:], mg_v[:])
        nc.vector.match_replace(mrep[:], mv8a[:], mg_v[:], NEG)
        nc.vector.max(mv8b[:], mrep[:])
        thr = pool.tile([128, 1], F32)
        nc.vector.tensor_copy(thr[:], mv8b[:, 1:2])
        mmask = pool.tile([128, 640], F32)
        nc.vector.tensor_scalar(mmask[:], mg_v[:], thr[:], scalar2=None,
                                op0=ALU.is_ge)
        mscan = pool.tile([128, 640], F32)
        nc.vector.tensor_tensor_scan(mscan[:], mmask[:], mmask[:], 0.0,
                                     op0=ALU.add, op1=ALU.bypass)
        nc.vector.tensor_mul(mscan[:], mscan[:], mmask[:])
        nc.vector.tensor_scalar(mscan[:], mscan[:], 1.0, scalar2=None,
                                op0=ALU.subtract)
        msel16 = pool.tile([128, 640], I16)
        nc.vector.tensor_copy(msel16[:], mscan[:])
        mg_i16 = pool.tile([128, 640], U16)
        nc.vector.tensor_copy(mg_i16[:], mg_i[:])
        tki16 = pool.tile([128, 16], U16)
        nc.gpsimd.local_scatter(tki16[:], mg_i16[:], msel16[:], channels=128,
                                num_elems=16, num_idxs=640)
        tki = pool.tile([128, 16], U32)
        nc.vector.tensor_copy(tki[:], tki16[:])

        # ---------------- m4 attention over top-10 ----------------
        qh = pool.tile([BL, D], F32)
        mmA(lambda j, w: qh[:, j:j + w], fzT, wq4T_d, bq4)
        s_att = pool.tile([128, H, KTOP], F32)
        vh_dram = dram.tile([128, KTOP, D], F32)
        EKTAGS = ("g3", "T_vis", "g1", "g2")
        KHTAGS = ("g6", "T_txt", "g5", "bigmid")
        VHTAGS = ("g7", "g4", "mo1", "ln_sq")
        for k0 in range(0, KTOP, 4):
            grp = [k for k in range(k0, min(k0 + 4, KTOP))]
            ekTs, khs, vhs = [], [], []
            for i, k in enumerate(grp):
                emb = pool.tile([128, D], F32, tag="embjit")
                nc.gpsimd.indirect_dma_start(
                    out=emb[:], out_offset=None, in_=ans_d,
                    in_offset=bass.IndirectOffsetOnAxis(ap=tki[:, k:k + 1],
                                                        axis=0))
                ekT = mid.tile([128, DK, BL], F32, tag=EKTAGS[i])
                for c in range(DK):
                    tps = psB.tile([128, BL], F32, tag="ps1")
                    nc.tensor.transpose(tps[:], emb[:, c * 128:(c + 1) * 128],
                                        ident[:])
                    nc.scalar.copy(ekT[:, c, :], tps[:])
                ekTs.append(ekT)
                kh_i = mid.tile([BL, D], F32, tag=KHTAGS[i])
                khs.append(kh_i)
                vh_i = mid.tile([BL, D], F32, tag=VHTAGS[i])
                vhs.append(vh_i)
            mmA_multi([(ekTs[i], (lambda j, w, _i=i: khs[_i][:, j:j + w]))
                       for i in range(len(grp))], wk4T_d, bk4)
            mmA_multi([(ekTs[i], (lambda j, w, _i=i: vhs[_i][:, j:j + w]))
                       for i in range(len(grp))], mha_wvT_d[4], bv4)
            for i, k in enumerate(grp):
                nc.sync.dma_start(vh_dram[:, k, :], vhs[i][:])
                prod = mid.tile([BL, D], F32, tag="g8")
                nc.vector.tensor_mul(prod[:], qh[:], khs[i][:])
                nc.vector.tensor_reduce(
                    s_att[:, :, k:k + 1].rearrange("p h k -> p (h k)"),
                    prod[:].rearrange("p (h d) -> p h d", h=H),
                    op=ALU.add, axis=mybir.AxisListType.X)

        smax = pool.tile([128, H], F32)
        nc.vector.tensor_reduce(smax[:], s_att[:], op=ALU.max,
                                axis=mybir.AxisListType.X)
        sexp = pool.tile([128, H, KTOP], F32)
        nc.vector.tensor_sub(
            sexp[:], s_att[:],
            smax[:].rearrange("p h -> p h ()").broadcast_to([128, H, KTOP]))
        nc.scalar.activation(sexp[:], sexp[:], AF.Exp,
                             scale=float(1.0 / np.sqrt(DH)))
        ssum = pool.tile([128, H], F32)
        nc.vector.tensor_reduce(ssum[:], sexp[:], op=ALU.add,
                                axis=mybir.AxisListType.X)
        srec = pool.tile([128, H], F32)
        nc.vector.reciprocal(srec[:], ssum[:])
        nc.vector.tensor_mul(
            sexp[:], sexp[:],
            srec[:].rearrange("p h -> p h ()").broadcast_to([128, H, KTOP]))
        o_nat = pool.tile([BL, D], F32)
        otmp = mid.tile([BL, D], F32, tag="g8")
        o3 = o_nat[:].rearrange("p (h d) -> p h d", h=H)
        t3 = otmp[:].rearrange("p (h d) -> p h d", h=H)
        for k in range(KTOP):
            att_b = sexp[:, :, k:k + 1].broadcast_to([128, H, DH])
            vh_k = mid.tile([BL, D], F32, tag="g7")
            nc.sync.dma_start(vh_k[:], vh_dram[:, k, :])
            v3 = vh_k[:].rearrange("p (h d) -> p h d", h=H)
            if k == 0:
                nc.vector.tensor_mul(o3, v3, att_b)
            else:
                nc.vector.tensor_mul(t3, v3, att_b)
                nc.vector.tensor_add(o_nat[:], o_nat[:], otmp[:])
        oT = mid.tile([128, DK, BL], F32, tag="g4")
        for c in range(DK):
            tps = psB.tile([128, BL], F32, tag="ps1")
            nc.tensor.transpose(tps[:], o_nat[:, c * 128:(c + 1) * 128],
                                ident[:])
            nc.scalar.copy(oT[:, c, :], tps[:])
        agT = mid.tile([128, DK, BL], F32, tag="mo1")
        mmT(agT, mha_outT_d[4], oT, DK, DK, bias_sb=mbias[:, 54:60])

        r3 = mid.tile([128, DK, BL], F32, tag="g4")
        nc.vector.tensor_add(r3[:], fzT[:], agT[:])
        fz2T = ln_T(r3, lng[:, 3 * DK:4 * DK], lnb[:, 3 * DK:4 * DK], "g5")
        h1T = mid.tile([128, 4 * DK, BL], F32, tag="bigmid")
        mmT(h1T, ffn1T_d, fz2T, DK, 4 * DK, bias_sb=biasp[:, 30:54],
            func=AF.Gelu)
        ffoT = mid.tile([128, DK, BL], F32, tag="g3")
        mmT(ffoT, ffn2T_d, h1T, 4 * DK, DK, bias_sb=biasp[:, 54:60])
        fz3T = mid.tile([128, DK, BL], F32, tag="g4")
        nc.vector.tensor_add(fz3T[:], fz2T[:], ffoT[:])
        outT = mid.tile([128, DK, BL], F32, tag="g5")
        mmT(outT, outpT_d, fz3T, DK, DK, bias_sb=biasp[:, 18:24])
        hidT_loc = mid.tile([128, DK, BL], F32, tag="g7")
        mmT(hidT_loc, open1T_d, outT, DK, DK, bias_sb=biasp[:, 24:30],
            func=AF.Gelu)

        # ---------------- AllGather hidden^T ----------------
        hag_in = dram.tile([128, DK * BL], F32)
        hag_out = dram.tile([NC, 128, DK * BL], F32)
        nc.gpsimd.dma_start(hag_in[:],
                            hidT_loc[:].rearrange("p c b -> p (c b)"))
        if fake_coll:
            for r in range(NC):
                nc.gpsimd.dma_start(hag_out[r], hag_in[:])
        else:
            nc.gpsimd.collective_compute(
                "AllGather", ALU.bypass, replica_groups=[list(range(NC))],
                ins=[hag_in.opt()], outs=[hag_out.opt()])
        hidT = big.tile([128, DK, B], F32, tag="actT_full")
        for k in range(DK):
            nc.sync.dma_start(
                hidT[:, k, :].rearrange("p (r b) -> p r b", r=NC),
                hag_out[:].rearrange("r p (c b) -> p c r b", c=DK)[:, k])

        # ---------------- open head ----------------
        for h in range(NQ):
            w2_sb = bankp.tile([128, DK, QW], F32, tag="bank_q")
            for k in range(DK):
                nc.sync.dma_start(
                    w2_sb[:, k, :],
                    w2T_d[k * 128:(k + 1) * 128, h * QW:(h + 1) * QW])
            b2c = pool.tile([1, QW], F32, tag="b2c")
            nc.sync.dma_start(b2c[:], b2_d[:, h * QW:(h + 1) * QW])
            for m in range(NC):
                outrow = sc.tile([128, QW], F32, tag="scores")
                for j in range(2):
                    w = NCHUNKS[j]; o = NCHOFF[j]
                    ps = psA.tile([128, 512], F32, tag="mm")
                    for k in range(DK):
                        nc.tensor.matmul(
                            ps[:, :w], hidT[:, k, m * BL:(m + 1) * BL],
                            w2_sb[:, k, o:o + w], start=(k == 0), stop=False)
                    nc.tensor.matmul(
                        ps[:, :w], ones_row[0:1, 0:BL],
                        b2c[:, o:o + w], start=False, stop=True)
                    nc.scalar.copy(outrow[:, o:o + w], ps[:, :w])
                for o3, w3 in ((0, 209), (209, 208), (417, 208)):
                    nc.sync.dma_start(
                        out_d[m * BL:(m + 1) * BL,
                              h * QW + o3:h * QW + o3 + w3],
                        outrow[:, o3:o3 + w3])
        es.close()

    nc.compile()
    return nc


# ======================= embedded SPMD runner =======================
class SpmdRunner:
    def __init__(self, nc, n_cores):
        import jax
        from jax.sharding import Mesh, PartitionSpec
        from jax.experimental.shard_map import shard_map
        from concourse.bass2jax import (_bass_exec_p, partition_id_tensor,
                                        install_neuronx_cc_hook)
        install_neuronx_cc_hook()
        self.jax = jax
        self.n_cores = n_cores
        pname = nc.partition_id_tensor.name if nc.partition_id_tensor else None
        in_names, out_names, out_avals, zero_outs = [], [], [], []
        for alloc in nc.m.functions[0].allocations:
            if not isinstance(alloc, mybir.MemoryLocationSet):
                continue
            name = alloc.memorylocations[0].name
            if alloc.kind == "ExternalInput":
                if name != pname:
                    in_names.append(name)
            elif alloc.kind == "ExternalOutput":
                out_names.append(name)
                shape = tuple(alloc.tensor_shape)
                dtype = mybir.dt.np(alloc.dtype)
                out_avals.append(jax.core.ShapedArray(shape, dtype))
                zero_outs.append(np.zeros(shape, dtype))
        self.in_names, self.out_names = in_names, out_names
        self.out_avals, self.zero_outs = out_avals, zero_outs
        n_params, n_outs = len(in_names), len(out_avals)
        all_in = in_names + out_names + ([pname] if pname else [])

        def _body(*args):
            operands = list(args)
            if pname is not None:
                operands.append(partition_id_tensor())
            outs = _bass_exec_p.bind(
                *operands, out_avals=tuple(out_avals), in_names=tuple(all_in),
                out_names=tuple(out_names), lowering_input_output_aliases=(),
                sim_require_finite=False, sim_require_nnan=False, nc=nc)
            return tuple(outs)

        devices = jax.devices()[:n_cores]
        self.mesh = Mesh(np.asarray(devices), ("core",))
        in_specs = (PartitionSpec("core"),) * (n_params + n_outs)
        out_specs = (PartitionSpec("core"),) * n_outs
        self.fn = jax.jit(
            shard_map(_body, mesh=self.mesh, in_specs=in_specs,
                      out_specs=out_specs, check_rep=False),
            keep_unused=True)
        self.PartitionSpec = PartitionSpec

    def stage(self, in_maps):
        jax, n = self.jax, self.n_cores
        per_core = [[np.asarray(in_maps[c][k]) for k in self.in_names]
                    for c in range(n)]
        concat_in = [np.concatenate([per_core[c][i] for c in range(n)], axis=0)
                     for i in range(len(self.in_names))]
        concat_zeros = [np.zeros((n * z.shape[0], *z.shape[1:]), z.dtype)
                        for z in self.zero_outs]
        sh = jax.sharding.NamedSharding(self.mesh, self.PartitionSpec("core"))
        self._staged = [jax.device_put(a, sh) for a in concat_in + concat_zeros]
        jax.block_until_ready(self._staged)

    def run(self):
        outs = self.fn(*self._staged)
        self.jax.block_until_ready(outs)
        return outs

    def results(self, outs):
        res = []
        for c in range(self.n_cores):
            d = {}
            for i, name in enumerate(self.out_names):
                a = np.asarray(outs[i])
                d[name] = a.reshape(self.n_cores, *self.out_avals[i].shape)[c]
            res.append(d)
        return res


_CACHE = {}


def _get_runner():
    if "runner" not in _CACHE:
        nc = build_program()
        _CACHE["runner"] = SpmdRunner(nc, NC)
    return _CACHE["runner"]


def kernel(**inputs):
    _lazy_imports()
    i = {k: np.ascontiguousarray(np.asarray(v, dtype=np.float32))
         for k, v in inputs.items()}
    T = lambda a: np.ascontiguousarray(a.T)
    mw, mb_ = i["mha_in_w"], i["mha_in_b"]
    ow, ob_ = i["mha_out_w"], i["mha_out_b"]

    def pack(dst, col, vec):
        n = vec.shape[0] // 128
        dst[:, col:col + n] = vec.reshape(n, 128).T

    biaspack = np.zeros((128, 72), np.float32)
    pack(biaspack, 0, i["vqa_out_b"]); pack(biaspack, 6, i["fproj_b"])
    pack(biaspack, 12, i["sim_b"]); pack(biaspack, 18, i["outp_b"])
    pack(biaspack, 24, i["open_b1"]); pack(biaspack, 30, i["ffn_b1"])
    pack(biaspack, 54, i["ffn_b2"])
    pack(biaspack, 66, i["vqa_in_b"][2 * D:3 * D])
    mbias = np.zeros((128, 60), np.float32)
    for q in range(5):
        pack(mbias, q * 12, mb_[q][2 * D:3 * D])
        pack(mbias, q * 12 + 6, ob_[q])
    lng = np.zeros((128, 4 * DK), np.float32); lnb = np.zeros_like(lng)
    for q in range(4):
        pack(lng, q * DK, i["ln_g"][q]); pack(lnb, q * DK, i["ln_b"][q])
    flng = np.zeros((128, DK), np.float32); flnb = np.zeros_like(flng)
    pack(flng, 0, i["fln_g"]); pack(flnb, 0, i["fln_b"])

    ansT = T(i["ans_emb"])
    w2T = T(i["open_w2"])
    shared = dict(
        ans=i["ans_emb"],
        vqa_wvT=T(i["vqa_in_w"][2 * D:3 * D]), vqa_outT=T(i["vqa_out_w"]),
        fprojT=T(i["fproj_w"]), simT=T(i["sim_w"]),
        wq4T=T(mw[4][:D]), wk4T=T(mw[4][D:2 * D]),
        ffn1T=T(i["ffn_w1"]), ffn2T=T(i["ffn_w2"]),
        outpT=T(i["outp_w"]), open1T=T(i["open_w1"]),
        biaspack=biaspack, mbiaspack=mbias, lng=lng, lnb=lnb,
        flng=flng, flnb=flnb,
        bq4=np.ascontiguousarray(mb_[4][:D].reshape(1, D)),
        bk4=np.ascontiguousarray(mb_[4][D:2 * D].reshape(1, D)),
        bv4=np.ascontiguousarray(mb_[4][2 * D:3 * D].reshape(1, D)),
    )
    for q in range(5):
        shared[f"m{q}_wvT"] = T(mw[q][2 * D:3 * D])
        shared[f"m{q}_outT"] = T(ow[q])

    in_maps = []
    for c in range(NC):
        noff = np.zeros((128, NQ), np.float32)
        for q in range(NQ):
            noff[:, q] = c * NS + q * QW
        m = dict(shared)
        m.update(
            vis=i["visual_feat"][c * BL:(c + 1) * BL],
            txt=i["text_feat"][c * BL:(c + 1) * BL],
            ansT=np.ascontiguousarray(ansT[:, c * NS:(c + 1) * NS]),
            w2T=np.ascontiguousarray(w2T[:, c * NS:(c + 1) * NS]),
            b2=np.ascontiguousarray(i["open_b2"][c * NS:(c + 1) * NS]
                                    .reshape(1, NS)),
            noff=noff,
        )
        in_maps.append(m)

    r = _get_runner()
    r.stage(in_maps)
    outs = r.run()
    res = r.results(outs)
    return np.concatenate([res[c]["out_slice"] for c in range(NC)], axis=1)

